# revision 20
# baseline (speedup 1.0000x reference)
"""Trainium2 Bass kernel for windowless relative-position-bias attention.

Problem (hardcoded shapes):
  x [16, 1024, 512] f32, W_qkv [512, 1536], rel_table [3969, 8],
  W_out [512, 512], b_out [512], rel_index [1048576] i32.

Sharding: pure data-parallel over batch -- core c owns batches (2c, 2c+1)
and computes all 8 heads for them locally. No collectives at all; each
core is fully independent (the old head-parallel scheme lost ~120us to
the end-of-kernel AllToAll + cross-core skew).

Device algorithm per core:
  - host pre-transposes x (xt [512, 2048] f16), pre-packs W_qkv, and
    precomputes exp(bias) per head (ebs [8*128, 8192] f16, streamed to
    SBUF per head, double buffered).
  - per batch: qkT blocks = W_qk^T x^T (blocks j<4 hold q head-pairs,
    j>=4 the k head-pairs, so qT_h and kT_h naturally share partition
    base 64*(h%2) -- matmul needs equal operand bases, not base 0);
    v in natural [keys, 512] layout with a ones column per head-block
    (the ones column makes the PV matmul also produce the softmax
    denominator).
  - per (batch, head): dots^T = kT^T qT per 128-key chunk (K=64);
    attn = exp(SCALE*dots) * exp_bias (exp on ACT, mult on DVE;
    softmax max-subtraction skipped -- logits bounded ~|7|);
    o_ps[65,1024] accumulates PV over chunks; denominator row 64 is
    PE-broadcast to 64 partitions, reciprocal on DVE, normalize-mult
    writes o2 [64 d, 1024 tok] f16 which is DMA-placed into the final
    lhsT layout (inner dim on partitions).
  - final projection per batch: out = o2_all^T @ W_out + b_out with
    K=128 full-rate matmuls; batch 0's projection is emitted before
    batch 1's attention so only batch 1's projection is tail.
"""

import os
import sys

for _p in ("/opt/trn_rl_repo", "/root/.axon_site/_ro/trn_rl_repo"):
    if os.path.isdir(_p) and _p not in sys.path:
        sys.path.insert(0, _p)

import numpy as np
import ml_dtypes

import concourse.bass as bass
import concourse.mybir as mybir
import concourse.tile as tile
from concourse import bacc
from concourse.bass import AP
from concourse.bass_utils import run_bass_kernel_spmd

# Content-hash NEFF cache: identical BIR -> reuse the compiled NEFF
# (neuronxcc is ~6 min; this makes repeat runs seconds).
import concourse.bass_utils as _bu
import concourse.bass2jax as _b2j

_orig_compile_bir = _bu.compile_bir_kernel


def _cached_compile_bir(bir_json, tmpdir, neff_name="file.neff"):
    import hashlib
    import shutil
    h = hashlib.sha256(bir_json).hexdigest()[:24]
    cdir = os.environ.get("NEFF_CACHE_DIR", "/tmp/neff_cache")
    os.makedirs(cdir, exist_ok=True)
    cpath = os.path.join(cdir, h + ".neff")
    if os.path.exists(cpath):
        dst = os.path.join(tmpdir, neff_name)
        shutil.copy(cpath, dst)
        return dst
    p = _orig_compile_bir(bir_json, tmpdir, neff_name)
    try:
        shutil.copy(p, cpath)
    except OSError:
        pass
    return p


_bu.compile_bir_kernel = _cached_compile_bir
_b2j.compile_bir_kernel = _cached_compile_bir

B, IH, IW = 16, 32, 32
N = IH * IW          # 1024
H, D = 8, 64
INNER = H * D        # 512
INP = OUP = 512
SCALE = D ** -0.5    # 0.125
NCORES = 8
BPC = B // NCORES    # batches per core = 2
TBL = (2 * IH - 1) * (2 * IW - 1)  # 3969

F32 = mybir.dt.float32
F32R = mybir.dt.float32r
BF16 = mybir.dt.bfloat16
F16 = mybir.dt.float16


def build_nc():
    nc = bacc.Bacc("TRN2", target_bir_lowering=False, num_devices=NCORES)

    xt_d = nc.dram_tensor("xt", [INP, BPC * N], F16, kind="ExternalInput")
    wqk_d = nc.dram_tensor("wqk", [INP, 2 * INNER], F16, kind="ExternalInput")
    wv_d = nc.dram_tensor("wv", [INP, INNER], F16, kind="ExternalInput")
    ebs_d = nc.dram_tensor("ebs", [H * 128, 8 * N], BF16, kind="ExternalInput")
    wout_d = nc.dram_tensor("wout", [INNER, OUP], BF16, kind="ExternalInput")
    bout_d = nc.dram_tensor("bout", [1, OUP], F32, kind="ExternalInput")
    biasb_d = nc.dram_tensor("biasb", [128, OUP], F32, kind="ExternalInput")
    ones_d = nc.dram_tensor("ones", [128], F32, kind="ExternalInput")
    out_d = nc.dram_tensor("out", [BPC * N, OUP], F32, kind="ExternalOutput")

    with tile.TileContext(nc) as tc:
        with (
            tc.tile_pool(name="consts", bufs=1) as consts,
            tc.tile_pool(name="xtp", bufs=2) as xtp,
            tc.tile_pool(name="qkp", bufs=2) as qkp,
            tc.tile_pool(name="vp", bufs=2) as vp,
            tc.tile_pool(name="ebp", bufs=2) as ebp,
            tc.tile_pool(name="attep", bufs=2) as attep,
            tc.tile_pool(name="attnp", bufs=3) as attnp,
            tc.tile_pool(name="dnp", bufs=2) as dnp,
            tc.tile_pool(name="rbp", bufs=2) as rbp,
            tc.tile_pool(name="o2p", bufs=2) as o2p,
            tc.tile_pool(name="lhsp", bufs=1) as lhsp,
            tc.tile_pool(name="outp", bufs=2) as outp,
            tc.tile_pool(name="psA", bufs=2, space="PSUM") as psA,
            tc.tile_pool(name="psO", bufs=2, space="PSUM") as psO,
        ):
            # ---- constants / weights to SBUF ----
            wqk_sb = consts.tile([128, 4, 2 * INNER], F16, tag="wqk")
            wv_sb = consts.tile([128, 4, INNER], F16, tag="wv")
            wout_sb = consts.tile([128, 4, OUP], BF16, tag="wout")
            bout_sb = consts.tile([65, OUP], F32R, tag="bout")
            bias_sb = consts.tile([128, OUP], F32, tag="biasb")
            ones1 = consts.tile([65, 128], F32R, tag="ones")
            for ic in range(4):
                nc.sync.dma_start(out=wqk_sb[:, ic, :], in_=wqk_d[ic * 128:(ic + 1) * 128, :])
            for ic in range(4):
                nc.scalar.dma_start(out=wv_sb[:, ic, :], in_=wv_d[ic * 128:(ic + 1) * 128, :])
                nc.scalar.dma_start(out=wout_sb[:, ic, :], in_=wout_d[ic * 128:(ic + 1) * 128, :])
            nc.scalar.dma_start(out=bout_sb[64:65, :], in_=bout_d[:].bitcast(F32R))
            nc.scalar.dma_start(out=bias_sb[:], in_=biasb_d[:])
            nc.scalar.dma_start(out=ones1[64:65, :], in_=ones_d[0:128].bitcast(F32R))

            lhs_all = lhsp.tile([128, 4, BPC * N], BF16, tag="lhs")

            # ---- per-batch qkv projection ----
            qk_tiles = {}
            va_tiles = {}
            for b in range(BPC):
                xt = xtp.tile([128, 4, N], F16, tag="xt")
                for ic in range(4):
                    nc.gpsimd.dma_start(
                        out=xt[:, ic, :],
                        in_=xt_d[ic * 128:(ic + 1) * 128, b * N:(b + 1) * N])

                qk = qkp.tile([128, 8, N], F32R, tag="qk")
                qk_tiles[b] = qk
                for blk in range(8):
                    qk_ps = psA.tile([128, N], F32, tag="big")
                    for fc in range(2):
                        for ic in range(4):
                            nc.tensor.matmul(
                                qk_ps[:, fc * 512:(fc + 1) * 512],
                                wqk_sb[:, ic, blk * 128:(blk + 1) * 128],
                                xt[:, ic, fc * 512:(fc + 1) * 512],
                                start=(ic == 0), stop=(ic == 3))
                    nc.scalar.copy(qk[:, blk, :], qk_ps[:])

                va = vp.tile([128, 8, H * (D + 1)], BF16, tag="va")
                va_tiles[b] = va
                for h in range(H):
                    nc.vector.memset(va[:, :, h * 65 + 64], 1.0)
                for kc in range(8):
                    v_ps = psA.tile([128, INNER], F32, tag="big")
                    for ic in range(4):
                        nc.tensor.matmul(
                            v_ps[:],
                            xt[:, ic, kc * 128:(kc + 1) * 128],
                            wv_sb[:, ic, :],
                            start=(ic == 0), stop=(ic == 3))
                    dst = va[:, kc, :].rearrange("p (h d) -> p h d", d=D + 1)[:, :, 0:D]
                    nc.vector.tensor_copy(dst, v_ps[:])

            # ---- attention units + final projections ----
            # The PE p-state drops to half clock on any queue gap over
            # ~900ns and takes a long busy stretch to recover, so the
            # normalize/drain of unit i (whose bcast matmul waits on an ACT
            # copy) is deferred into the middle of unit i+1, and final(b0)
            # row-blocks are interleaved between b1's units: the PE queue
            # never sits on an instruction whose inputs aren't ready yet.
            pending = [None]

            def drain_stage(stage):
                # drain of unit i-1, staged across the early slots of unit
                # i so no engine ever waits on a freshly-produced input:
                #   0: raw-denominator row copy to SBUF f32r (ACT/DVE alternating)
                #   1: PE-broadcast of the denominator to 64 partitions
                #   2: wide reciprocal (1-partition DVE ops are slow)
                #   3: normalize-mult + placement DMA into lhs_all
                if pending[0] is None:
                    return
                p = pending[0]
                if stage == 0:
                    p["dn"] = dnp.tile([D + 1, N], F32R, tag="dn", name="dn")
                    if (p["b"] * H + p["h"]) % 2 == 0:
                        nc.scalar.copy(p["dn"][D:D + 1, :],
                                       p["o_ps"][D:D + 1, :])
                    else:
                        nc.vector.tensor_copy(p["dn"][D:D + 1, :],
                                              p["o_ps"][D:D + 1, :])
                elif stage == 1:
                    p["rb_ps"] = psA.tile([D, N], F32, tag="big", name="rb_ps")
                    for fc in range(2):
                        nc.tensor.matmul(
                            p["rb_ps"][:, fc * 512:(fc + 1) * 512],
                            ones1[64:65, 0:D],
                            p["dn"][D:D + 1, fc * 512:(fc + 1) * 512],
                            start=True, stop=True)
                elif stage == 2:
                    p["rb"] = rbp.tile([D, N], F32, tag="rb", name="rb")
                    nc.vector.reciprocal_approx_fast(p["rb"][:], p["rb_ps"][:])
                else:
                    b, h, o_ps = p["b"], p["h"], p["o_ps"]
                    pb = 64 * (h % 2)
                    o2b = o2p.tile([D, N], BF16, tag="o2b")
                    nc.vector.tensor_tensor(o2b[:], o_ps[0:D, :], p["rb"][:],
                                            mybir.AluOpType.mult)
                    nc.sync.dma_start(
                        out=lhs_all[pb:pb + 64, h // 2, b * N:(b + 1) * N],
                        in_=o2b[:])
                    pending[0] = None

            def drain_all():
                for s in range(4):
                    drain_stage(s)

            def unit(b, h, eb, prefetch=None):
                qk = qk_tiles[b]
                va = va_tiles[b]
                pb = 64 * (h % 2)
                qT = qk[pb:pb + 64, h // 2, :]
                kT = qk[pb:pb + 64, 4 + h // 2, :]
                o_ps = psO.tile([D + 1, N], F32, tag="ops")
                attn_tiles = {}

                def dots_stage(mc):
                    dots_ps = psA.tile([128, N], F32, tag="big")
                    for fc in range(2):
                        nc.tensor.matmul(
                            dots_ps[:, fc * 512:(fc + 1) * 512],
                            kT[:, mc * 128:(mc + 1) * 128],
                            qT[:, fc * 512:(fc + 1) * 512],
                            start=True, stop=True)
                    attn_e = attep.tile([128, N], BF16, tag="attn_e")
                    nc.scalar.activation(attn_e[:], dots_ps[:],
                                         mybir.ActivationFunctionType.Exp,
                                         scale=SCALE)
                    attn = attnp.tile([128, N], BF16, tag="attn")
                    nc.vector.tensor_tensor(attn[:], attn_e[:], eb[:, mc, :],
                                            mybir.AluOpType.mult)
                    attn_tiles[mc] = attn

                def pv_stage(mc):
                    attn = attn_tiles.pop(mc)
                    for fc in range(2):
                        nc.tensor.matmul(
                            o_ps[:, fc * 512:(fc + 1) * 512],
                            va[:, mc, h * 65:(h + 1) * 65],
                            attn[:, fc * 512:(fc + 1) * 512],
                            start=(mc == 0), stop=(mc == 7))

                # PV lags dots by 2 slots so the PE queue never blocks on
                # the exp->mult chain of the same chunk
                drain_stage(0)
                dots_stage(0)
                dots_stage(1)
                pv_stage(0)
                if prefetch is not None:
                    prefetch()
                dots_stage(2)
                drain_stage(1)
                pv_stage(1)
                drain_stage(2)
                dots_stage(3)
                pv_stage(2)
                drain_stage(3)
                for mc in range(4, 8):
                    dots_stage(mc)
                    pv_stage(mc - 1)
                pv_stage(7)
                pending[0] = {"b": b, "h": h, "o_ps": o_ps}

            def final_nq(b, nq):
                r0 = b * N + nq * 128
                ps_f = psA.tile([128, OUP], F32, tag="big")
                for kc in range(4):
                    nc.tensor.matmul(
                        ps_f[:],
                        lhs_all[:, kc, r0:r0 + 128],
                        wout_sb[:, kc, :],
                        start=(kc == 0), stop=(kc == 3))
                o_sb = outp.tile([128, OUP], F32, tag="osb")
                nc.vector.tensor_tensor(o_sb[:], ps_f[:], bias_sb[:],
                                        mybir.AluOpType.add)
                nc.sync.dma_start(out=out_d[r0:r0 + 128, :], in_=o_sb[:])

            # batch 0 heads ascending, batch 1 descending (reuses the last
            # eb tile); final(0) row-blocks interleave between b1's units;
            # eb tiles are prefetched one unit ahead (mid-unit) so the
            # first bias-mult of a unit never waits on HBM
            seq = [(0, h) for h in range(H)] + [(1, h) for h in range(H)]
            eb_tiles = {}
            eb_order = []

            def load_eb(h, eng=None):
                # mirror the bufs=3 pool ring: only the last 3 loaded heads
                # have live tiles
                if h in eb_tiles:
                    return
                t = ebp.tile([128, 8, N], BF16, tag="eb")
                (eng or nc.gpsimd).dma_start(out=t[:],
                                             in_=ebs_d[h * 128:(h + 1) * 128, :])
                eb_tiles[h] = t
                eb_order.append(h)
                if len(eb_order) > 2:
                    del eb_tiles[eb_order.pop(0)]

            load_eb(seq[0][1], eng=nc.scalar)
            for j, (b, h) in enumerate(seq):
                nxt = seq[j + 1][1] if j + 1 < len(seq) else None

                def prefetch(nxt=nxt):
                    if nxt is not None:
                        load_eb(nxt)

                unit(b, h, eb_tiles[h], prefetch=prefetch)
                if b == 1:
                    final_nq(0, h)
            drain_all()
            for nq in range(8):
                final_nq(1, nq)

    nc.finalize()
    return nc


_NC_CACHE = None


def _get_nc():
    global _NC_CACHE
    if _NC_CACHE is None:
        _NC_CACHE = build_nc()
    return _NC_CACHE


def make_in_maps(x, W_qkv, rel_table, W_out, b_out, rel_index):
    x = np.asarray(x, np.float32)
    W_qkv = np.asarray(W_qkv, np.float32)
    W_out = np.asarray(W_out, np.float32)
    b_out = np.asarray(b_out, np.float32).reshape(1, OUP)
    rel_table = np.asarray(rel_table, np.float32)
    rel_index = np.asarray(rel_index, np.int64)

    wqk = np.ascontiguousarray(W_qkv[:, :2 * INNER]).astype(np.float16)
    wv = np.ascontiguousarray(W_qkv[:, 2 * INNER:]).astype(np.float16)
    wout = W_out.astype(ml_dtypes.bfloat16)

    # exp(bias) in the transposed orientation the device consumes:
    # ebs[h, p, mc, t] = exp(bias_ref[h, t, mc*128 + p])
    bias_full = rel_table[rel_index].reshape(N, N, H)     # [t?, m?, h] = [i, j, h]
    ebT = np.exp(bias_full).transpose(2, 1, 0)            # [h, m, t]
    ebs = np.ascontiguousarray(
        ebT.reshape(H, 8, 128, N).transpose(0, 2, 1, 3)   # [h, p, mc, t]
    ).astype(ml_dtypes.bfloat16).reshape(H * 128, 8 * N)

    ones = np.ones(128, np.float32)
    biasb = np.ascontiguousarray(np.broadcast_to(b_out, (128, OUP))).astype(np.float32)

    in_maps = []
    for c in range(NCORES):
        xt = np.ascontiguousarray(
            x[BPC * c:BPC * (c + 1)].reshape(BPC * N, INP).T).astype(np.float16)
        in_maps.append({
            "xt": xt, "wqk": wqk, "wv": wv, "ebs": ebs,
            "wout": wout, "bout": b_out, "ones": ones,
            "biasb": biasb,
        })
    return in_maps


def run(inputs, trace=False, **kw):
    nc = _get_nc()
    in_maps = make_in_maps(inputs["x"], inputs["W_qkv"], inputs["rel_table"],
                           inputs["W_out"], inputs["b_out"],
                           inputs["rel_index"])
    res = run_bass_kernel_spmd(nc, in_maps, core_ids=list(range(NCORES)),
                               trace=trace, **kw)
    out = np.empty((B, N, OUP), np.float32)
    for c in range(NCORES):
        out[BPC * c:BPC * (c + 1)] = res.results[c]["out"].reshape(BPC, N, OUP)
    return out, res


def kernel(**inputs):
    out, _ = run(inputs, trace=False)
    return out


# revision 23
# speedup vs baseline: 1.0529x; 1.0529x over previous
"""Trainium2 Bass kernel for windowless relative-position-bias attention.

Problem (hardcoded shapes):
  x [16, 1024, 512] f32, W_qkv [512, 1536], rel_table [3969, 8],
  W_out [512, 512], b_out [512], rel_index [1048576] i32.

Sharding: pure data-parallel over batch -- core c owns batches (2c, 2c+1)
and computes all 8 heads for them locally. No collectives at all; each
core is fully independent (the old head-parallel scheme lost ~120us to
the end-of-kernel AllToAll + cross-core skew).

Device algorithm per core:
  - host pre-transposes x (xt [512, 2048] f16), pre-packs W_qkv, and
    precomputes exp(bias) per head (ebs [8*128, 8192] f16, streamed to
    SBUF per head, double buffered).
  - per batch: qkT blocks = W_qk^T x^T (blocks j<4 hold q head-pairs,
    j>=4 the k head-pairs, so qT_h and kT_h naturally share partition
    base 64*(h%2) -- matmul needs equal operand bases, not base 0);
    v in natural [keys, 512] layout with a ones column per head-block
    (the ones column makes the PV matmul also produce the softmax
    denominator).
  - per (batch, head): dots^T = kT^T qT per 128-key chunk (K=64);
    attn = exp(SCALE*dots) * exp_bias (exp on ACT, mult on DVE;
    softmax max-subtraction skipped -- logits bounded ~|7|);
    o_ps[65,1024] accumulates PV over chunks; denominator row 64 is
    PE-broadcast to 64 partitions, reciprocal on DVE, normalize-mult
    writes o2 [64 d, 1024 tok] f16 which is DMA-placed into the final
    lhsT layout (inner dim on partitions).
  - final projection per batch: out = o2_all^T @ W_out + b_out with
    K=128 full-rate matmuls; batch 0's projection is emitted before
    batch 1's attention so only batch 1's projection is tail.
"""

import os
import sys

for _p in ("/opt/trn_rl_repo", "/root/.axon_site/_ro/trn_rl_repo"):
    if os.path.isdir(_p) and _p not in sys.path:
        sys.path.insert(0, _p)

import numpy as np
import ml_dtypes

import concourse.bass as bass
import concourse.mybir as mybir
import concourse.tile as tile
from concourse import bacc
from concourse.bass import AP
from concourse.bass_utils import run_bass_kernel_spmd

# Content-hash NEFF cache: identical BIR -> reuse the compiled NEFF
# (neuronxcc is ~6 min; this makes repeat runs seconds).
import concourse.bass_utils as _bu
import concourse.bass2jax as _b2j

_orig_compile_bir = _bu.compile_bir_kernel


def _cached_compile_bir(bir_json, tmpdir, neff_name="file.neff"):
    import hashlib
    import shutil
    h = hashlib.sha256(bir_json).hexdigest()[:24]
    cdir = os.environ.get("NEFF_CACHE_DIR", "/tmp/neff_cache")
    os.makedirs(cdir, exist_ok=True)
    cpath = os.path.join(cdir, h + ".neff")
    if os.path.exists(cpath):
        dst = os.path.join(tmpdir, neff_name)
        shutil.copy(cpath, dst)
        return dst
    p = _orig_compile_bir(bir_json, tmpdir, neff_name)
    try:
        shutil.copy(p, cpath)
    except OSError:
        pass
    return p


_bu.compile_bir_kernel = _cached_compile_bir
_b2j.compile_bir_kernel = _cached_compile_bir

B, IH, IW = 16, 32, 32
N = IH * IW          # 1024
H, D = 8, 64
INNER = H * D        # 512
INP = OUP = 512
SCALE = D ** -0.5    # 0.125
NCORES = 8
BPC = B // NCORES    # batches per core = 2
TBL = (2 * IH - 1) * (2 * IW - 1)  # 3969

F32 = mybir.dt.float32
F32R = mybir.dt.float32r
BF16 = mybir.dt.bfloat16
F16 = mybir.dt.float16


def build_nc():
    nc = bacc.Bacc("TRN2", target_bir_lowering=False, num_devices=NCORES)

    xt_d = nc.dram_tensor("xt", [INP, BPC * N], F16, kind="ExternalInput")
    wqk_d = nc.dram_tensor("wqk", [INP, 2 * INNER], F16, kind="ExternalInput")
    wv_d = nc.dram_tensor("wv", [INP, INNER], F16, kind="ExternalInput")
    ebs_d = nc.dram_tensor("ebs", [H * 128, 8 * N], BF16, kind="ExternalInput")
    wout_d = nc.dram_tensor("wout", [INNER, OUP], BF16, kind="ExternalInput")
    biasb_d = nc.dram_tensor("biasb", [128, OUP], F32, kind="ExternalInput")
    out_d = nc.dram_tensor("out", [BPC * N, OUP], F32, kind="ExternalOutput")

    with tile.TileContext(nc) as tc:
        with (
            tc.tile_pool(name="consts", bufs=1) as consts,
            tc.tile_pool(name="xtp", bufs=2) as xtp,
            tc.tile_pool(name="qkp", bufs=2) as qkp,
            tc.tile_pool(name="vp", bufs=2) as vp,
            tc.tile_pool(name="ebp", bufs=2) as ebp,
            tc.tile_pool(name="attep", bufs=2) as attep,
            tc.tile_pool(name="attnp", bufs=3) as attnp,
            tc.tile_pool(name="dnp", bufs=2) as dnp,
            tc.tile_pool(name="rbp", bufs=2) as rbp,
            tc.tile_pool(name="o2p", bufs=2) as o2p,
            tc.tile_pool(name="lhsp", bufs=1) as lhsp,
            tc.tile_pool(name="outp", bufs=2) as outp,
            tc.tile_pool(name="psA", bufs=2, space="PSUM") as psA,
            tc.tile_pool(name="psO", bufs=2, space="PSUM") as psO,
        ):
            # ---- constants / weights to SBUF ----
            wqk_sb = consts.tile([128, 4, 2 * INNER], F16, tag="wqk")
            wv_sb = consts.tile([128, 4, INNER], F16, tag="wv")
            wout_sb = consts.tile([128, 4, OUP], BF16, tag="wout")
            bias_sb = consts.tile([128, OUP], F32, tag="biasb")
            for ic in range(4):
                nc.sync.dma_start(out=wqk_sb[:, ic, :], in_=wqk_d[ic * 128:(ic + 1) * 128, :])
            for ic in range(4):
                nc.scalar.dma_start(out=wv_sb[:, ic, :], in_=wv_d[ic * 128:(ic + 1) * 128, :])
                nc.scalar.dma_start(out=wout_sb[:, ic, :], in_=wout_d[ic * 128:(ic + 1) * 128, :])
            nc.scalar.dma_start(out=bias_sb[:], in_=biasb_d[:])

            lhs_all = lhsp.tile([128, 4, BPC * N], BF16, tag="lhs")

            # ---- per-batch qkv projection ----
            qk_tiles = {}
            va_tiles = {}
            for b in range(BPC):
                xt = xtp.tile([128, 4, N], F16, tag="xt")
                for ic in range(4):
                    nc.gpsimd.dma_start(
                        out=xt[:, ic, :],
                        in_=xt_d[ic * 128:(ic + 1) * 128, b * N:(b + 1) * N])

                qk = qkp.tile([128, 8, N], BF16, tag="qk")
                qk_tiles[b] = qk
                for blk in range(8):
                    qk_ps = psA.tile([128, N], F32, tag="big")
                    for fc in range(2):
                        for ic in range(4):
                            nc.tensor.matmul(
                                qk_ps[:, fc * 512:(fc + 1) * 512],
                                wqk_sb[:, ic, blk * 128:(blk + 1) * 128],
                                xt[:, ic, fc * 512:(fc + 1) * 512],
                                start=(ic == 0), stop=(ic == 3))
                    nc.scalar.copy(qk[:, blk, :], qk_ps[:])

                va = vp.tile([128, 8, H * (D + 1)], BF16, tag="va")
                va_tiles[b] = va
                for h in range(H):
                    nc.vector.memset(va[:, :, h * 65 + 64], 1.0)
                for kc in range(8):
                    v_ps = psA.tile([128, INNER], F32, tag="big")
                    for ic in range(4):
                        nc.tensor.matmul(
                            v_ps[:],
                            xt[:, ic, kc * 128:(kc + 1) * 128],
                            wv_sb[:, ic, :],
                            start=(ic == 0), stop=(ic == 3))
                    dst = va[:, kc, :].rearrange("p (h d) -> p h d", d=D + 1)[:, :, 0:D]
                    nc.vector.tensor_copy(dst, v_ps[:])

            # ---- attention units + final projections ----
            # The PE p-state drops to half clock on any queue gap over
            # ~900ns and takes a long busy stretch to recover, so the
            # normalize/drain of unit i (whose bcast matmul waits on an ACT
            # copy) is deferred into the middle of unit i+1, and final(b0)
            # row-blocks are interleaved between b1's units: the PE queue
            # never sits on an instruction whose inputs aren't ready yet.
            pending = [None]

            def drain_stage(stage):
                # drain of unit i-1, staged across the early slots of unit
                # i so no engine ever waits on a freshly-produced input:
                #   0: raw-denominator row copy to SBUF f32r (ACT/DVE alternating)
                #   1: PE-broadcast of the denominator to 64 partitions
                #   2: wide reciprocal (1-partition DVE ops are slow)
                #   3: normalize-mult + placement DMA into lhs_all
                if pending[0] is None:
                    return
                p = pending[0]
                if stage == 0:
                    p["dn"] = dnp.tile([D + 1, N], F32, tag="dn", name="dn")
                    nc.scalar.copy(p["dn"][D:D + 1, :], p["o_ps"][D:D + 1, :])
                elif stage == 1:
                    p["dn0"] = dnp.tile([1, N], F32, tag="dn0", name="dn0")
                    nc.sync.dma_start(out=p["dn0"][:], in_=p["dn"][D:D + 1, :])
                elif stage == 2:
                    p["rbraw"] = rbp.tile([D, N], F32, tag="rbraw",
                                          name="rbraw")
                    nc.gpsimd.partition_broadcast(p["rbraw"][:], p["dn0"][:],
                                                  channels=D)
                elif stage == 3:
                    p["rb"] = rbp.tile([D, N], F32, tag="rb", name="rb")
                    nc.vector.reciprocal_approx_fast(p["rb"][:], p["rbraw"][:])
                else:
                    b, h, o_ps = p["b"], p["h"], p["o_ps"]
                    pb = 64 * (h % 2)
                    o2b = o2p.tile([D, N], BF16, tag="o2b")
                    nc.vector.tensor_tensor(o2b[:], o_ps[0:D, :], p["rb"][:],
                                            mybir.AluOpType.mult)
                    nc.sync.dma_start(
                        out=lhs_all[pb:pb + 64, h // 2, b * N:(b + 1) * N],
                        in_=o2b[:])
                    pending[0] = None

            def drain_all():
                for s in range(5):
                    drain_stage(s)

            def unit(b, h, eb, prefetch=None):
                qk = qk_tiles[b]
                va = va_tiles[b]
                pb = 64 * (h % 2)
                qT = qk[pb:pb + 64, h // 2, :]
                kT = qk[pb:pb + 64, 4 + h // 2, :]
                o_ps = psO.tile([D + 1, N], F32, tag="ops")
                attn_tiles = {}

                def dots_stage(mc):
                    dots_ps = psA.tile([128, N], F32, tag="big")
                    for fc in range(2):
                        nc.tensor.matmul(
                            dots_ps[:, fc * 512:(fc + 1) * 512],
                            kT[:, mc * 128:(mc + 1) * 128],
                            qT[:, fc * 512:(fc + 1) * 512],
                            start=True, stop=True)
                    attn_e = attep.tile([128, N], BF16, tag="attn_e")
                    nc.scalar.activation(attn_e[:], dots_ps[:],
                                         mybir.ActivationFunctionType.Exp,
                                         scale=SCALE)
                    attn = attnp.tile([128, N], BF16, tag="attn")
                    nc.vector.tensor_tensor(attn[:], attn_e[:], eb[:, mc, :],
                                            mybir.AluOpType.mult)
                    attn_tiles[mc] = attn

                def pv_stage(mc):
                    attn = attn_tiles.pop(mc)
                    for fc in range(2):
                        nc.tensor.matmul(
                            o_ps[:, fc * 512:(fc + 1) * 512],
                            va[:, mc, h * 65:(h + 1) * 65],
                            attn[:, fc * 512:(fc + 1) * 512],
                            start=(mc == 0), stop=(mc == 7))

                # PV lags dots by 2 slots so the PE queue never blocks on
                # the exp->mult chain of the same chunk
                drain_stage(0)
                dots_stage(0)
                drain_stage(1)
                dots_stage(1)
                pv_stage(0)
                if prefetch is not None:
                    prefetch()
                dots_stage(2)
                drain_stage(2)
                pv_stage(1)
                drain_stage(3)
                dots_stage(3)
                pv_stage(2)
                drain_stage(4)
                for mc in range(4, 8):
                    dots_stage(mc)
                    pv_stage(mc - 1)
                pv_stage(7)
                pending[0] = {"b": b, "h": h, "o_ps": o_ps}

            def final_nq(b, nq):
                r0 = b * N + nq * 128
                ps_f = psA.tile([128, OUP], F32, tag="big")
                for kc in range(4):
                    nc.tensor.matmul(
                        ps_f[:],
                        lhs_all[:, kc, r0:r0 + 128],
                        wout_sb[:, kc, :],
                        start=(kc == 0), stop=(kc == 3))
                o_sb = outp.tile([128, OUP], F32, tag="osb")
                nc.vector.tensor_tensor(o_sb[:], ps_f[:], bias_sb[:],
                                        mybir.AluOpType.add)
                nc.sync.dma_start(out=out_d[r0:r0 + 128, :], in_=o_sb[:])

            # batch 0 heads ascending, batch 1 descending (reuses the last
            # eb tile); final(0) row-blocks interleave between b1's units;
            # eb tiles are prefetched one unit ahead (mid-unit) so the
            # first bias-mult of a unit never waits on HBM
            seq = [(0, h) for h in range(H)] + [(1, h) for h in range(H)]
            eb_tiles = {}
            eb_order = []

            def load_eb(h, eng=None):
                # mirror the bufs=3 pool ring: only the last 3 loaded heads
                # have live tiles
                if h in eb_tiles:
                    return
                t = ebp.tile([128, 8, N], BF16, tag="eb")
                (eng or nc.gpsimd).dma_start(out=t[:],
                                             in_=ebs_d[h * 128:(h + 1) * 128, :])
                eb_tiles[h] = t
                eb_order.append(h)
                if len(eb_order) > 2:
                    del eb_tiles[eb_order.pop(0)]

            load_eb(seq[0][1], eng=nc.scalar)
            for j, (b, h) in enumerate(seq):
                nxt = seq[j + 1][1] if j + 1 < len(seq) else None

                def prefetch(nxt=nxt):
                    if nxt is not None:
                        load_eb(nxt)

                unit(b, h, eb_tiles[h], prefetch=prefetch)
                if b == 1:
                    final_nq(0, h)
            drain_all()
            for nq in range(8):
                final_nq(1, nq)

    nc.finalize()
    return nc


_NC_CACHE = None


def _get_nc():
    global _NC_CACHE
    if _NC_CACHE is None:
        _NC_CACHE = build_nc()
    return _NC_CACHE


def make_in_maps(x, W_qkv, rel_table, W_out, b_out, rel_index):
    x = np.asarray(x, np.float32)
    W_qkv = np.asarray(W_qkv, np.float32)
    W_out = np.asarray(W_out, np.float32)
    b_out = np.asarray(b_out, np.float32).reshape(1, OUP)
    rel_table = np.asarray(rel_table, np.float32)
    rel_index = np.asarray(rel_index, np.int64)

    wqk = np.ascontiguousarray(W_qkv[:, :2 * INNER]).astype(np.float16)
    wv = np.ascontiguousarray(W_qkv[:, 2 * INNER:]).astype(np.float16)
    wout = W_out.astype(ml_dtypes.bfloat16)

    # exp(bias) in the transposed orientation the device consumes:
    # ebs[h, p, mc, t] = exp(bias_ref[h, t, mc*128 + p])
    bias_full = rel_table[rel_index].reshape(N, N, H)     # [t?, m?, h] = [i, j, h]
    ebT = np.exp(bias_full).transpose(2, 1, 0)            # [h, m, t]
    ebs = np.ascontiguousarray(
        ebT.reshape(H, 8, 128, N).transpose(0, 2, 1, 3)   # [h, p, mc, t]
    ).astype(ml_dtypes.bfloat16).reshape(H * 128, 8 * N)

    biasb = np.ascontiguousarray(np.broadcast_to(b_out, (128, OUP))).astype(np.float32)

    in_maps = []
    for c in range(NCORES):
        xt = np.ascontiguousarray(
            x[BPC * c:BPC * (c + 1)].reshape(BPC * N, INP).T).astype(np.float16)
        in_maps.append({
            "xt": xt, "wqk": wqk, "wv": wv, "ebs": ebs,
            "wout": wout, "biasb": biasb,
        })
    return in_maps


def run(inputs, trace=False, **kw):
    nc = _get_nc()
    in_maps = make_in_maps(inputs["x"], inputs["W_qkv"], inputs["rel_table"],
                           inputs["W_out"], inputs["b_out"],
                           inputs["rel_index"])
    res = run_bass_kernel_spmd(nc, in_maps, core_ids=list(range(NCORES)),
                               trace=trace, **kw)
    out = np.empty((B, N, OUP), np.float32)
    for c in range(NCORES):
        out[BPC * c:BPC * (c + 1)] = res.results[c]["out"].reshape(BPC, N, OUP)
    return out, res


def kernel(**inputs):
    out, _ = run(inputs, trace=False)
    return out


# revision 25
# speedup vs baseline: 1.0617x; 1.0083x over previous
"""Trainium2 Bass kernel for windowless relative-position-bias attention.

Problem (hardcoded shapes):
  x [16, 1024, 512] f32, W_qkv [512, 1536], rel_table [3969, 8],
  W_out [512, 512], b_out [512], rel_index [1048576] i32.

Sharding: pure data-parallel over batch -- core c owns batches (2c, 2c+1)
and computes all 8 heads for them locally. No collectives at all; each
core is fully independent (the old head-parallel scheme lost ~120us to
the end-of-kernel AllToAll + cross-core skew).

Device algorithm per core:
  - host pre-transposes x (xt [512, 2048] f16), pre-packs W_qkv, and
    precomputes exp(bias) per head (ebs [8*128, 8192] f16, streamed to
    SBUF per head, double buffered).
  - per batch: qkT blocks = W_qk^T x^T (blocks j<4 hold q head-pairs,
    j>=4 the k head-pairs, so qT_h and kT_h naturally share partition
    base 64*(h%2) -- matmul needs equal operand bases, not base 0);
    v in natural [keys, 512] layout with a ones column per head-block
    (the ones column makes the PV matmul also produce the softmax
    denominator).
  - per (batch, head): dots^T = kT^T qT per 128-key chunk (K=64);
    attn = exp(SCALE*dots) * exp_bias (exp on ACT, mult on DVE;
    softmax max-subtraction skipped -- logits bounded ~|7|);
    o_ps[65,1024] accumulates PV over chunks; denominator row 64 is
    PE-broadcast to 64 partitions, reciprocal on DVE, normalize-mult
    writes o2 [64 d, 1024 tok] f16 which is DMA-placed into the final
    lhsT layout (inner dim on partitions).
  - final projection per batch: out = o2_all^T @ W_out + b_out with
    K=128 full-rate matmuls; batch 0's projection is emitted before
    batch 1's attention so only batch 1's projection is tail.
"""

import os
import sys

for _p in ("/opt/trn_rl_repo", "/root/.axon_site/_ro/trn_rl_repo"):
    if os.path.isdir(_p) and _p not in sys.path:
        sys.path.insert(0, _p)

import numpy as np
import ml_dtypes

import concourse.bass as bass
import concourse.mybir as mybir
import concourse.tile as tile
from concourse import bacc
from concourse.bass import AP
from concourse.bass_utils import run_bass_kernel_spmd

# Content-hash NEFF cache: identical BIR -> reuse the compiled NEFF
# (neuronxcc is ~6 min; this makes repeat runs seconds).
import concourse.bass_utils as _bu
import concourse.bass2jax as _b2j

_orig_compile_bir = _bu.compile_bir_kernel


def _cached_compile_bir(bir_json, tmpdir, neff_name="file.neff"):
    import hashlib
    import shutil
    h = hashlib.sha256(bir_json).hexdigest()[:24]
    cdir = os.environ.get("NEFF_CACHE_DIR", "/tmp/neff_cache")
    os.makedirs(cdir, exist_ok=True)
    cpath = os.path.join(cdir, h + ".neff")
    if os.path.exists(cpath):
        dst = os.path.join(tmpdir, neff_name)
        shutil.copy(cpath, dst)
        return dst
    p = _orig_compile_bir(bir_json, tmpdir, neff_name)
    try:
        shutil.copy(p, cpath)
    except OSError:
        pass
    return p


_bu.compile_bir_kernel = _cached_compile_bir
_b2j.compile_bir_kernel = _cached_compile_bir

B, IH, IW = 16, 32, 32
N = IH * IW          # 1024
H, D = 8, 64
INNER = H * D        # 512
INP = OUP = 512
SCALE = D ** -0.5    # 0.125
NCORES = 8
BPC = B // NCORES    # batches per core = 2
TBL = (2 * IH - 1) * (2 * IW - 1)  # 3969

F32 = mybir.dt.float32
F32R = mybir.dt.float32r
BF16 = mybir.dt.bfloat16
F16 = mybir.dt.float16


def build_nc():
    nc = bacc.Bacc("TRN2", target_bir_lowering=False, num_devices=NCORES)

    xt_d = nc.dram_tensor("xt", [INP, BPC * N], F16, kind="ExternalInput")
    wqk_d = nc.dram_tensor("wqk", [INP, 2 * INNER], F16, kind="ExternalInput")
    wv_d = nc.dram_tensor("wv", [INP, INNER], F16, kind="ExternalInput")
    ebs_d = nc.dram_tensor("ebs", [H * 128, 8 * N], BF16, kind="ExternalInput")
    wout_d = nc.dram_tensor("wout", [INNER, OUP], BF16, kind="ExternalInput")
    biasb_d = nc.dram_tensor("biasb", [128, OUP], F32, kind="ExternalInput")
    ones_d = nc.dram_tensor("ones", [128], F32, kind="ExternalInput")
    out_d = nc.dram_tensor("out", [BPC * N, OUP], F32, kind="ExternalOutput")

    with tile.TileContext(nc) as tc:
        with (
            tc.tile_pool(name="consts", bufs=1) as consts,
            tc.tile_pool(name="xtp", bufs=2) as xtp,
            tc.tile_pool(name="qkp", bufs=2) as qkp,
            tc.tile_pool(name="vp", bufs=2) as vp,
            tc.tile_pool(name="ebp", bufs=2) as ebp,
            tc.tile_pool(name="attep", bufs=2) as attep,
            tc.tile_pool(name="attnp", bufs=3) as attnp,
            tc.tile_pool(name="dnp", bufs=2) as dnp,
            tc.tile_pool(name="rbp", bufs=2) as rbp,
            tc.tile_pool(name="o2p", bufs=2) as o2p,
            tc.tile_pool(name="lhsp", bufs=1) as lhsp,
            tc.tile_pool(name="outp", bufs=2) as outp,
            tc.tile_pool(name="psA", bufs=2, space="PSUM") as psA,
            tc.tile_pool(name="psO", bufs=2, space="PSUM") as psO,
        ):
            # ---- constants / weights to SBUF ----
            wqk_sb = consts.tile([128, 4, 2 * INNER], F16, tag="wqk")
            wv_sb = consts.tile([128, 4, INNER], F16, tag="wv")
            wout_sb = consts.tile([128, 4, OUP], BF16, tag="wout")
            bias_sb = consts.tile([128, OUP], F32, tag="biasb")
            ones1 = consts.tile([65, 128], F32R, tag="ones")
            for ic in range(4):
                nc.sync.dma_start(out=wqk_sb[:, ic, :], in_=wqk_d[ic * 128:(ic + 1) * 128, :])
            for ic in range(4):
                nc.scalar.dma_start(out=wv_sb[:, ic, :], in_=wv_d[ic * 128:(ic + 1) * 128, :])
                nc.scalar.dma_start(out=wout_sb[:, ic, :], in_=wout_d[ic * 128:(ic + 1) * 128, :])
            nc.scalar.dma_start(out=bias_sb[:], in_=biasb_d[:])
            nc.scalar.dma_start(out=ones1[64:65, :], in_=ones_d[0:128].bitcast(F32R))

            lhs_all = lhsp.tile([128, 4, BPC * N], BF16, tag="lhs")

            # ---- per-batch qkv projection ----
            qk_tiles = {}
            va_tiles = {}
            for b in range(BPC):
                xt = xtp.tile([128, 4, N], F16, tag="xt")
                for ic in range(4):
                    nc.gpsimd.dma_start(
                        out=xt[:, ic, :],
                        in_=xt_d[ic * 128:(ic + 1) * 128, b * N:(b + 1) * N])

                qk = qkp.tile([128, 8, N], BF16, tag="qk")
                qk_tiles[b] = qk
                for blk in range(8):
                    qk_ps = psA.tile([128, N], F32, tag="big")
                    for fc in range(2):
                        for ic in range(4):
                            nc.tensor.matmul(
                                qk_ps[:, fc * 512:(fc + 1) * 512],
                                wqk_sb[:, ic, blk * 128:(blk + 1) * 128],
                                xt[:, ic, fc * 512:(fc + 1) * 512],
                                start=(ic == 0), stop=(ic == 3))
                    nc.scalar.copy(qk[:, blk, :], qk_ps[:])

                va = vp.tile([128, 8, H * (D + 1)], BF16, tag="va")
                va_tiles[b] = va
                for h in range(H):
                    nc.vector.memset(va[:, :, h * 65 + 64], 1.0)
                for kc in range(8):
                    v_ps = psA.tile([128, INNER], F32, tag="big")
                    for ic in range(4):
                        nc.tensor.matmul(
                            v_ps[:],
                            xt[:, ic, kc * 128:(kc + 1) * 128],
                            wv_sb[:, ic, :],
                            start=(ic == 0), stop=(ic == 3))
                    dst = va[:, kc, :].rearrange("p (h d) -> p h d", d=D + 1)[:, :, 0:D]
                    nc.vector.tensor_copy(dst, v_ps[:])

            # ---- attention units + final projections ----
            # The PE p-state drops to half clock on any queue gap over
            # ~900ns and takes a long busy stretch to recover, so the
            # normalize/drain of unit i (whose bcast matmul waits on an ACT
            # copy) is deferred into the middle of unit i+1, and final(b0)
            # row-blocks are interleaved between b1's units: the PE queue
            # never sits on an instruction whose inputs aren't ready yet.
            pending = [None]

            def drain_stage(stage):
                # drain of unit i-1, staged across the early slots of unit
                # i so no engine ever waits on a freshly-produced input:
                #   0: raw-denominator row copy to SBUF f32r (ACT/DVE alternating)
                #   1: PE-broadcast of the denominator to 64 partitions
                #   2: wide reciprocal (1-partition DVE ops are slow)
                #   3: normalize-mult + placement DMA into lhs_all
                if pending[0] is None:
                    return
                p = pending[0]
                if stage == 0:
                    p["dn"] = dnp.tile([D + 1, N], F32, tag="dn", name="dn")
                    nc.scalar.copy(p["dn"][D:D + 1, :], p["o_ps"][D:D + 1, :])
                elif stage == 1:
                    p["dn0"] = dnp.tile([1, N], F32, tag="dn0", name="dn0")
                    nc.sync.dma_start(out=p["dn0"][:], in_=p["dn"][D:D + 1, :])
                elif stage == 2:
                    p["rbraw"] = rbp.tile([D, N], F32, tag="rbraw",
                                          name="rbraw")
                    nc.gpsimd.partition_broadcast(p["rbraw"][:], p["dn0"][:],
                                                  channels=D)
                elif stage == 3:
                    p["rb"] = rbp.tile([D, N], F32, tag="rb", name="rb")
                    nc.vector.reciprocal_approx_fast(p["rb"][:], p["rbraw"][:])
                else:
                    b, h, o_ps = p["b"], p["h"], p["o_ps"]
                    pb = 64 * (h % 2)
                    o2b = o2p.tile([D, N], BF16, tag="o2b")
                    nc.vector.tensor_tensor(o2b[:], o_ps[0:D, :], p["rb"][:],
                                            mybir.AluOpType.mult)
                    nc.sync.dma_start(
                        out=lhs_all[pb:pb + 64, h // 2, b * N:(b + 1) * N],
                        in_=o2b[:])
                    pending[0] = None

            def drain_all():
                # tail-only variant: PE broadcast (PE is idle at the tail;
                # the DMA-shift + Pool-broadcast chain is ~7us longer)
                if pending[0] is None:
                    return
                p = pending[0]
                b, h, o_ps = p["b"], p["h"], p["o_ps"]
                pb = 64 * (h % 2)
                dnr = dnp.tile([D + 1, N], F32R, tag="dnr", name="dnr")
                nc.scalar.copy(dnr[D:D + 1, :], o_ps[D:D + 1, :])
                rb_ps = psA.tile([D, N], F32, tag="big", name="rb_ps")
                for fc in range(2):
                    nc.tensor.matmul(
                        rb_ps[:, fc * 512:(fc + 1) * 512],
                        ones1[64:65, 0:D],
                        dnr[D:D + 1, fc * 512:(fc + 1) * 512],
                        start=True, stop=True)
                rb = rbp.tile([D, N], F32, tag="rb", name="rb")
                nc.vector.reciprocal_approx_fast(rb[:], rb_ps[:])
                o2b = o2p.tile([D, N], BF16, tag="o2b")
                nc.vector.tensor_tensor(o2b[:], o_ps[0:D, :], rb[:],
                                        mybir.AluOpType.mult)
                nc.sync.dma_start(
                    out=lhs_all[pb:pb + 64, h // 2, b * N:(b + 1) * N],
                    in_=o2b[:])
                pending[0] = None

            def unit(b, h, eb, prefetch=None):
                qk = qk_tiles[b]
                va = va_tiles[b]
                pb = 64 * (h % 2)
                qT = qk[pb:pb + 64, h // 2, :]
                kT = qk[pb:pb + 64, 4 + h // 2, :]
                o_ps = psO.tile([D + 1, N], F32, tag="ops")
                attn_tiles = {}

                def dots_stage(mc):
                    dots_ps = psA.tile([128, N], F32, tag="big")
                    for fc in range(2):
                        nc.tensor.matmul(
                            dots_ps[:, fc * 512:(fc + 1) * 512],
                            kT[:, mc * 128:(mc + 1) * 128],
                            qT[:, fc * 512:(fc + 1) * 512],
                            start=True, stop=True)
                    attn_e = attep.tile([128, N], BF16, tag="attn_e")
                    nc.scalar.activation(attn_e[:], dots_ps[:],
                                         mybir.ActivationFunctionType.Exp,
                                         scale=SCALE)
                    attn = attnp.tile([128, N], BF16, tag="attn")
                    nc.vector.tensor_tensor(attn[:], attn_e[:], eb[:, mc, :],
                                            mybir.AluOpType.mult)
                    attn_tiles[mc] = attn

                def pv_stage(mc):
                    attn = attn_tiles.pop(mc)
                    for fc in range(2):
                        nc.tensor.matmul(
                            o_ps[:, fc * 512:(fc + 1) * 512],
                            va[:, mc, h * 65:(h + 1) * 65],
                            attn[:, fc * 512:(fc + 1) * 512],
                            start=(mc == 0), stop=(mc == 7))

                # PV lags dots by 2 slots so the PE queue never blocks on
                # the exp->mult chain of the same chunk
                drain_stage(0)
                dots_stage(0)
                drain_stage(1)
                dots_stage(1)
                pv_stage(0)
                drain_stage(2)
                if prefetch is not None:
                    prefetch()
                dots_stage(2)
                drain_stage(3)
                pv_stage(1)
                drain_stage(4)
                dots_stage(3)
                pv_stage(2)
                for mc in range(4, 8):
                    dots_stage(mc)
                    pv_stage(mc - 1)
                pv_stage(7)
                pending[0] = {"b": b, "h": h, "o_ps": o_ps}

            def final_nq(b, nq):
                r0 = b * N + nq * 128
                ps_f = psA.tile([128, OUP], F32, tag="big")
                for kc in range(4):
                    nc.tensor.matmul(
                        ps_f[:],
                        lhs_all[:, kc, r0:r0 + 128],
                        wout_sb[:, kc, :],
                        start=(kc == 0), stop=(kc == 3))
                o_sb = outp.tile([128, OUP], F32, tag="osb")
                nc.vector.tensor_tensor(o_sb[:], ps_f[:], bias_sb[:],
                                        mybir.AluOpType.add)
                nc.sync.dma_start(out=out_d[r0:r0 + 128, :], in_=o_sb[:])

            # batch 0 heads ascending, batch 1 descending (reuses the last
            # eb tile); final(0) row-blocks interleave between b1's units;
            # eb tiles are prefetched one unit ahead (mid-unit) so the
            # first bias-mult of a unit never waits on HBM
            seq = [(0, h) for h in range(H)] + [(1, h) for h in range(H)]
            eb_tiles = {}
            eb_order = []

            def load_eb(h, eng=None):
                # mirror the bufs=3 pool ring: only the last 3 loaded heads
                # have live tiles
                if h in eb_tiles:
                    return
                t = ebp.tile([128, 8, N], BF16, tag="eb")
                (eng or nc.gpsimd).dma_start(out=t[:],
                                             in_=ebs_d[h * 128:(h + 1) * 128, :])
                eb_tiles[h] = t
                eb_order.append(h)
                if len(eb_order) > 2:
                    del eb_tiles[eb_order.pop(0)]

            load_eb(seq[0][1], eng=nc.scalar)
            for j, (b, h) in enumerate(seq):
                nxt = seq[j + 1][1] if j + 1 < len(seq) else None

                def prefetch(nxt=nxt):
                    if nxt is not None:
                        load_eb(nxt)

                unit(b, h, eb_tiles[h], prefetch=prefetch)
                if b == 1:
                    final_nq(0, h)
            drain_all()
            for nq in range(8):
                final_nq(1, nq)

    nc.finalize()
    return nc


_NC_CACHE = None


def _get_nc():
    global _NC_CACHE
    if _NC_CACHE is None:
        _NC_CACHE = build_nc()
    return _NC_CACHE


def make_in_maps(x, W_qkv, rel_table, W_out, b_out, rel_index):
    x = np.asarray(x, np.float32)
    W_qkv = np.asarray(W_qkv, np.float32)
    W_out = np.asarray(W_out, np.float32)
    b_out = np.asarray(b_out, np.float32).reshape(1, OUP)
    rel_table = np.asarray(rel_table, np.float32)
    rel_index = np.asarray(rel_index, np.int64)

    wqk = np.ascontiguousarray(W_qkv[:, :2 * INNER]).astype(np.float16)
    wv = np.ascontiguousarray(W_qkv[:, 2 * INNER:]).astype(np.float16)
    wout = W_out.astype(ml_dtypes.bfloat16)

    # exp(bias) in the transposed orientation the device consumes:
    # ebs[h, p, mc, t] = exp(bias_ref[h, t, mc*128 + p])
    bias_full = rel_table[rel_index].reshape(N, N, H)     # [t?, m?, h] = [i, j, h]
    ebT = np.exp(bias_full).transpose(2, 1, 0)            # [h, m, t]
    ebs = np.ascontiguousarray(
        ebT.reshape(H, 8, 128, N).transpose(0, 2, 1, 3)   # [h, p, mc, t]
    ).astype(ml_dtypes.bfloat16).reshape(H * 128, 8 * N)

    biasb = np.ascontiguousarray(np.broadcast_to(b_out, (128, OUP))).astype(np.float32)

    in_maps = []
    for c in range(NCORES):
        xt = np.ascontiguousarray(
            x[BPC * c:BPC * (c + 1)].reshape(BPC * N, INP).T).astype(np.float16)
        in_maps.append({
            "xt": xt, "wqk": wqk, "wv": wv, "ebs": ebs,
            "wout": wout, "biasb": biasb, "ones": np.ones(128, np.float32),
        })
    return in_maps


def run(inputs, trace=False, **kw):
    nc = _get_nc()
    in_maps = make_in_maps(inputs["x"], inputs["W_qkv"], inputs["rel_table"],
                           inputs["W_out"], inputs["b_out"],
                           inputs["rel_index"])
    res = run_bass_kernel_spmd(nc, in_maps, core_ids=list(range(NCORES)),
                               trace=trace, **kw)
    out = np.empty((B, N, OUP), np.float32)
    for c in range(NCORES):
        out[BPC * c:BPC * (c + 1)] = res.results[c]["out"].reshape(BPC, N, OUP)
    return out, res


def kernel(**inputs):
    out, _ = run(inputs, trace=False)
    return out
